# revision 10
# baseline (speedup 1.0000x reference)
"""Trainium2 Bass kernel for nn_EpiGraphModel (2x GATv2 + BN/ELU + residual
proj + 2-layer LSTM + FC), data-parallel over the 16 (b,t) graphs on 8
NeuronCores (2 graphs/core). The LSTM batch (b*n) splits along the same
graph boundary, so there is no cross-core communication.

Self-contained: hardcodes shapes; host numpy does index/weight folding;
device code is bass/Tile run via run_bass_kernel_spmd.

Math restructuring (validated vs reference at ~3e-7 rel err in fp32):
  * logit_k = 0.2(p_s+q_d) + 0.8*sum_h att_kh*relu(z_h): node matmuls
    premultiply columns with 0.8*att (sign included), so the edge reduce is
    sum(max(z,0)) over positive-att cols + sum(min(z,0)) over negative-att
    cols (DVE tensor_scalar with accum_out). The 0.2(p+q) terms ride as two
    extra gathered columns pqA/pqB placed at the ends of the two slices:
    max(u,0)+min(u,0) = u recovers them exactly.
  * no segment-max (logits are O(1)); den and message aggregation via
    one-hot S^T matmuls over dst-sorted 128-edge chunks (PSUM accumulate).
  * sum_e w*z'' = sum_e w*xl''[s] + den_d*xr''[d]  =>  gat_out =
    AGG/(0.8*att*den) - xr_true, folded into the BN affine; BN applied
    during the PE-transpose eviction to channel-major layout.
  * ELU(z) = max(z, exp(min(z,0)) - 1).
  * per-edge z'' built by two indirect-DMA row gathers (src rows, then dst
    rows with CCE accumulate-add); bf16 DRAM staging, cast to f32 on gather.
"""

import numpy as np
import ml_dtypes

import concourse.bass as bass
import concourse.bacc as bacc
import concourse.tile as tile
from concourse import mybir
from concourse.bass import IndirectOffsetOnAxis
from concourse.bass_utils import run_bass_kernel_spmd
from concourse.masks import make_identity

B, T, N, F_IN = 2, 8, 1000, 64
HID, HEADS, E = 128, 2, 16000
NG = B * T
SEQ = N // T
P = 128
N_CORES = 8
GPC = NG // N_CORES

F32 = mybir.dt.float32
BF16 = mybir.dt.bfloat16
I32 = mybir.dt.int32
AF = mybir.ActivationFunctionType
OP = mybir.AluOpType

# fallback switches (flip if a fast path breaks on sim/hw)
CCE_ADD = True        # fuse xl+xr via DMA accumulate on the 2nd gather
CAST_GATHER = True    # bf16 staging cast to f32 during gather


# ===================================================================== host
def _build_edges(edge_index):
    src = np.asarray(edge_index[0], np.int64)
    dst = np.asarray(edge_index[1], np.int64)
    order = np.argsort(dst, kind="stable")
    src_s, dst_s = src[order], dst[order]

    src_cols, dst_cols, s_blocks, win_nch = [], [], [], []
    n_win = (N + P - 1) // P
    for w in range(n_win):
        lo, hi = w * P, min((w + 1) * P, N)
        sel = (dst_s >= lo) & (dst_s < hi)
        e_src, e_dst = src_s[sel], dst_s[sel]
        ne = len(e_src)
        nch = max(1, (ne + P - 1) // P)
        win_nch.append(nch)
        for c in range(nch):
            a, b = c * P, min((c + 1) * P, ne)
            k = b - a
            si = np.full(P, N, np.int32)
            di = np.full(P, N, np.int32)
            S = np.zeros((P, P), np.float32)
            si[:k] = e_src[a:b]
            di[:k] = e_dst[a:b]
            S[np.arange(k), e_dst[a:b] - lo] = 1.0
            src_cols.append(si)
            dst_cols.append(di)
            s_blocks.append(S)
    return dict(
        src_idx=np.ascontiguousarray(np.stack(src_cols, 1)),
        dst_idx=np.ascontiguousarray(np.stack(dst_cols, 1)),
        S=np.ascontiguousarray(np.concatenate(s_blocks, 1)),
        win_nch=win_nch, n_win=n_win, nch=len(s_blocks),
    )


def _fold_gat(Wl, Wr, att, b_gat, g, be, m, v):
    K, H = att.shape
    CB = H + 2
    perm, n_pos = [], []
    for k in range(K):
        pos = np.where(att[k] >= 0)[0]
        neg = np.where(att[k] < 0)[0]
        n_pos.append(len(pos))
        perm += [k * H + h for h in pos] + [k * H + h for h in neg]
    perm = np.array(perm, np.int64)
    scale = 0.8 * att.reshape(-1)[perm]

    Fin = Wl.shape[0]
    Ml = np.zeros((Fin, K * CB), np.float32)
    Mr = np.zeros((Fin, K * CB), np.float32)
    for k in range(K):
        s = k * CB
        pq_l = 0.2 * (Wl[:, k * H:(k + 1) * H] @ att[k])
        pq_r = 0.2 * (Wr[:, k * H:(k + 1) * H] @ att[k])
        cols = perm[k * H:(k + 1) * H]
        sc = scale[k * H:(k + 1) * H]
        Ml[:, s] = pq_l
        Mr[:, s] = pq_r
        Ml[:, s + 1:s + 1 + H] = Wl[:, cols] * sc[None, :]
        Mr[:, s + 1:s + 1 + H] = Wr[:, cols] * sc[None, :]
        Ml[:, s + 1 + H] = pq_l
        Mr[:, s + 1 + H] = pq_r

    gg = g / np.sqrt(v + 1e-5)
    G = (gg[perm] / scale).astype(np.float32)
    beta = ((b_gat - m) * gg + be)[perm].astype(np.float32)
    slc = [(k * CB, k * CB + 1 + n_pos[k], (k + 1) * CB) for k in range(K)]
    return dict(Ml=Ml, Mr=Mr, G=G, beta=beta, perm=perm, slc=slc,
                K=K, H=H, CB=CB, CT=K * CB)


def _fold_all(p):
    f1 = _fold_gat(p["g1_Wl"], p["g1_Wr"], p["g1_att"], p["g1_b"],
                   p["bn1_g"], p["bn1_b"], p["bn1_m"], p["bn1_v"])
    f2 = _fold_gat(p["g2_Wl"], p["g2_Wr"], p["g2_att"], p["g2_b"],
                   p["bn2_g"], p["bn2_b"], p["bn2_m"], p["bn2_v"])
    p1, p2 = f1["perm"], f2["perm"]
    f2 = dict(f2, Ml=f2["Ml"][p1], Mr=f2["Mr"][p1])
    return dict(
        f1=f1, f2=f2,
        projW=np.ascontiguousarray(p["proj_W"][:, p2].astype(np.float32)),
        projb=p["proj_b"][p2].astype(np.float32).reshape(P, 1),
        Wih0T=np.ascontiguousarray(p["lstm0_Wih"].T[p2].astype(np.float32)),
        Whh0T=np.ascontiguousarray(p["lstm0_Whh"].T.astype(np.float32)),
        bias0=np.ascontiguousarray(
            (p["lstm0_bih"] + p["lstm0_bhh"]).astype(np.float32).reshape(4, P).T),
        Wih1T=np.ascontiguousarray(p["lstm1_Wih"].T.astype(np.float32)),
        Whh1T=np.ascontiguousarray(p["lstm1_Whh"].T.astype(np.float32)),
        bias1=np.ascontiguousarray(
            (p["lstm1_bih"] + p["lstm1_bhh"]).astype(np.float32).reshape(4, P).T),
        fc1W=np.ascontiguousarray(p["fc1_W"].astype(np.float32)),
        fc1b=p["fc1_b"].astype(np.float32).reshape(64, 1),
        fc2W=np.ascontiguousarray(p["fc2_W"].astype(np.float32)),
        fc2b=np.asarray(p["fc2_b"], np.float32).reshape(1, 1),
    )


# =================================================================== device
# per-layer channel config; slc recomputed per fold at runtime, but chunk/
# window structure and slice boundaries are baked into the trace, so the
# trace must be rebuilt if att sign patterns change (kernel() handles this
# by keying the cache on the fold-derived slice boundaries too).
def _build_program(ed, slc1, slc2):
    nc = bacc.Bacc("TRN2", target_bir_lowering=False, debug=False,
                   num_devices=N_CORES)
    NCH = ed["nch"]
    STG_DT = BF16 if CAST_GATHER else F32

    def din(name, shape, dt):
        return nc.dram_tensor(name, shape, dt, kind="ExternalInput").ap()

    xT_d = din("xT", [64, GPC, N], F32)
    Ml_d = {0: din("Ml1", [64, 1, 260], F32), 1: din("Ml2", [P, 2, 130], F32)}
    Mr_d = {0: din("Mr1", [64, 1, 260], F32), 1: din("Mr2", [P, 2, 130], F32)}
    G_d = {0: din("G1", [P, 2], F32), 1: din("G2", [P, 1], F32)}
    beta_d = {0: din("beta1", [P, 2], F32), 1: din("beta2", [P, 1], F32)}
    projW_d = din("projW", [64, P], F32)
    projb_d = din("projb", [P, 1], F32)
    Wih_d = {0: din("Wih0T", [P, 512], F32), 1: din("Wih1T", [P, 512], F32)}
    Whh_d = {0: din("Whh0T", [P, 512], F32), 1: din("Whh1T", [P, 512], F32)}
    b_d = {0: din("bias0", [P, 4], F32), 1: din("bias1", [P, 4], F32)}
    fc1W_d = din("fc1W", [P, 64], F32)
    fc1b_d = din("fc1b", [64, 1], F32)
    fc2W_d = din("fc2W", [64, 1], F32)
    fc2b_d = din("fc2b", [1, 1], F32)
    srci_d = din("src_idx", [P, NCH], I32)
    dsti_d = din("dst_idx", [P, NCH], I32)
    S_d = din("S", [P, NCH * P], BF16)
    out_d = nc.dram_tensor("out", [1, GPC * SEQ], F32,
                           kind="ExternalOutput").ap()

    LCH = {0: dict(CT=260, K=2, nct=2, slc=slc1),
           1: dict(CT=130, K=1, nct=1, slc=slc2)}
    stg = {}
    for g in range(GPC):
        for l in (0, 1):
            for s in range(2):
                stg[(g, l, s)] = nc.dram_tensor(
                    f"stg_{g}_{l}_{s}", [N + 1, LCH[l]["CT"]], STG_DT,
                    kind="Internal").ap()

    with tile.TileContext(nc) as tc:
        with (
            tc.tile_pool(name="const", bufs=1) as cp,
            tc.tile_pool(name="state", bufs=1) as st,
            tc.tile_pool(name="big", bufs=1) as bg,
            tc.tile_pool(name="work", bufs=3) as wk,
            tc.tile_pool(name="zt", bufs=max(ed["win_nch"]) + 3) as zp,
            tc.tile_pool(name="msgp", bufs=4) as mp,
            tc.tile_pool(name="lstm", bufs=2) as lp,
        ):
            def load(name, dram, shape, dt):
                t = cp.tile(shape, dt, tag=name)
                nc.sync.dma_start(t[:], dram)
                return t

            xT = load("xT", xT_d, [64, GPC, N], F32)
            Ml = {l: load(f"Ml{l}", Ml_d[l], list(Ml_d[l].shape), F32)
                  for l in (0, 1)}
            Mr = {l: load(f"Mr{l}", Mr_d[l], list(Mr_d[l].shape), F32)
                  for l in (0, 1)}
            G = {l: load(f"G{l}", G_d[l], list(G_d[l].shape), F32)
                 for l in (0, 1)}
            beta = {l: load(f"beta{l}", beta_d[l], list(beta_d[l].shape), F32)
                    for l in (0, 1)}
            projW = load("projW", projW_d, [64, P], F32)
            projb = load("projb", projb_d, [P, 1], F32)
            WihT = {l: load(f"Wih{l}", Wih_d[l], [P, 512], F32) for l in (0, 1)}
            WhhT = {l: load(f"Whh{l}", Whh_d[l], [P, 512], F32) for l in (0, 1)}
            bias = {l: load(f"bias{l}", b_d[l], [P, 4], F32) for l in (0, 1)}
            fc1W = load("fc1W", fc1W_d, [P, 64], F32)
            fc1b = load("fc1b", fc1b_d, [64, 1], F32)
            fc2W = load("fc2W", fc2W_d, [64, 1], F32)
            fc2b = load("fc2b", fc2b_d, [1, 1], F32)
            srci = load("srci", srci_d, [P, NCH], I32)
            dsti = load("dsti", dsti_d, [P, NCH], I32)
            S_sb = load("S", S_d, [P, NCH * P], BF16)

            ident = cp.tile([P, P], F32, tag="ident")
            make_identity(nc, ident[:])

            zrow = cp.tile([1, 260], STG_DT, tag="zrow")
            nc.vector.memset(zrow[:], 0.0)
            for (g, l, s), d in stg.items():
                ct = LCH[l]["CT"]
                nc.sync.dma_start(d[N:N + 1, :], zrow[:1, :ct])

            spatT = st.tile([P, GPC, N], F32, tag="spatT")

            gat_ps = tc.tile_pool(name="gatps", bufs=1, space="PSUM")
            psA = psB = psT = gat_ps.__enter__()
            for g in range(GPC):
                h_in = None  # layer input, set per layer
                for l in (0, 1):
                    cfg = LCH[l]
                    CT, K, nct, slc = cfg["CT"], cfg["K"], cfg["nct"], cfg["slc"]
                    kdims = [64] if l == 0 else [P, P]

                    def lhs_in(kt, lo, hi):
                        if l == 0:
                            return xT[:64, g, lo:hi]
                        return h_in[:, kt, lo:hi]

                    # ---- node matmuls, row orientation -> bf16 DRAM staging
                    for side, M_w in ((0, Ml[l]), (1, Mr[l])):
                        for m in range(8):
                            mlo, mhi = m * P, min(N, (m + 1) * P)
                            mw = mhi - mlo
                            ps = psA.tile([P, CT], F32, tag="node", bufs=2)
                            for kt, kd in enumerate(kdims):
                                nc.tensor.matmul(
                                    ps[:mw, :], lhsT=lhs_in(kt, mlo, mhi),
                                    rhs=M_w[:kd, kt, :],
                                    start=(kt == 0), stop=(kt == len(kdims) - 1))
                            row = wk.tile([P, CT], STG_DT, tag="rowst")
                            nc.vector.tensor_copy(row[:mw, :], ps[:mw, :])
                            nc.sync.dma_start(
                                stg[(g, l, side)][mlo:mhi, :], row[:mw, :])

                    # ---- xr'' in T orientation -> X = xr''*G - beta
                    XT = bg.tile([P, nct, N], F32, tag=f"XT{l}")
                    for ct in range(nct):
                        for nh in range(2):
                            lo, hi = nh * (N // 2), (nh + 1) * (N // 2)
                            ps = psB.tile([P, N // 2], F32, tag="xrT")
                            for kt, kd in enumerate(kdims):
                                nc.tensor.matmul(
                                    ps[:],
                                    lhsT=Mr[l][:kd, kt,
                                               ct * 130 + 1:ct * 130 + 129],
                                    rhs=lhs_in(kt, lo, hi),
                                    start=(kt == 0), stop=(kt == len(kdims) - 1))
                            nc.vector.tensor_scalar(
                                out=XT[:, ct, lo:hi], in0=ps[:],
                                scalar1=G[l][:, ct:ct + 1],
                                scalar2=beta[l][:, ct:ct + 1],
                                op0=OP.mult, op1=OP.subtract)

                    # ---- edge phase
                    zbn = bg.tile([P, nct, N], F32, tag=f"zbn{l}")
                    cbase = 0
                    for w, nchw in enumerate(ed["win_nch"]):
                        wlo = w * P
                        wlen = min(N, wlo + P) - wlo
                        accs = wk.tile([P, 2, nchw, K], F32, tag="accs")
                        zts = []
                        for c in range(nchw):
                            ci = cbase + c
                            z = zp.tile([P, CT], F32, tag="z")
                            nc.gpsimd.indirect_dma_start(
                                out=z[:], out_offset=None,
                                in_=stg[(g, l, 0)],
                                in_offset=IndirectOffsetOnAxis(
                                    ap=srci[:, ci:ci + 1], axis=0))
                            if CCE_ADD:
                                nc.gpsimd.indirect_dma_start(
                                    out=z[:], out_offset=None,
                                    in_=stg[(g, l, 1)],
                                    in_offset=IndirectOffsetOnAxis(
                                        ap=dsti[:, ci:ci + 1], axis=0),
                                    compute_op=OP.add)
                            else:
                                z2 = zp.tile([P, CT], F32, tag="z2")
                                nc.gpsimd.indirect_dma_start(
                                    out=z2[:], out_offset=None,
                                    in_=stg[(g, l, 1)],
                                    in_offset=IndirectOffsetOnAxis(
                                        ap=dsti[:, ci:ci + 1], axis=0))
                                nc.vector.tensor_tensor(
                                    out=z[:], in0=z[:], in1=z2[:], op=OP.add)
                            for k, (s0, mid, e0) in enumerate(slc):
                                scr = wk.tile([P, 130], F32, tag="scr")
                                scr2 = wk.tile([P, 130], F32, tag="scr2")
                                nc.vector.tensor_scalar(
                                    out=scr[:, :mid - s0], in0=z[:, s0:mid],
                                    scalar1=0.0, scalar2=None,
                                    op0=OP.max, op1=OP.add,
                                    accum_out=accs[:, 0, c, k:k + 1])
                                nc.vector.tensor_scalar(
                                    out=scr2[:, :e0 - mid], in0=z[:, mid:e0],
                                    scalar1=0.0, scalar2=None,
                                    op0=OP.min, op1=OP.add,
                                    accum_out=accs[:, 1, c, k:k + 1])
                            zts.append(z)

                        lg = wk.tile([P, nchw, K], F32, tag="lg")
                        nc.vector.tensor_tensor(
                            out=lg[:], in0=accs[:, 0], in1=accs[:, 1],
                            op=OP.add)
                        wt = wk.tile([P, nchw, K], F32, tag="wt")
                        nc.scalar.activation(wt[:], lg[:], AF.Exp)
                        wb = wk.tile([P, nchw, K], BF16, tag="wb")
                        nc.vector.tensor_copy(wb[:], wt[:])

                        agg = psA.tile([P, nct * P], F32, tag="agg", bufs=2)
                        den = psB.tile([P, K], F32, tag="den")
                        for c in range(nchw):
                            z = zts[c]
                            msg = mp.tile([P, nct * P], BF16, tag="msg")
                            for k in range(K):
                                nc.vector.tensor_scalar(
                                    out=msg[:, k * P:(k + 1) * P],
                                    in0=z[:, k * 130 + 1:k * 130 + 129],
                                    scalar1=wt[:, c, k:k + 1], scalar2=None,
                                    op0=OP.mult)
                            Sc = S_sb[:, (cbase + c) * P:(cbase + c + 1) * P]
                            nc.tensor.matmul(agg[:], lhsT=Sc, rhs=msg[:],
                                             start=(c == 0), stop=(c == nchw - 1))
                            nc.tensor.matmul(den[:], lhsT=Sc, rhs=wb[:, c, :],
                                             start=(c == 0), stop=(c == nchw - 1))

                        denr = wk.tile([P, K], F32, tag="denr")
                        nc.vector.tensor_scalar(
                            out=denr[:], in0=den[:], scalar1=1e-16,
                            scalar2=None, op0=OP.add)
                        rd = wk.tile([P, K], F32, tag="rd")
                        nc.vector.reciprocal(rd[:], denr[:])
                        A = wk.tile([P, nct * P], F32, tag="Arow")
                        for k in range(K):
                            nc.vector.tensor_scalar(
                                out=A[:, k * P:(k + 1) * P],
                                in0=agg[:, k * P:(k + 1) * P],
                                scalar1=rd[:, k:k + 1], scalar2=None,
                                op0=OP.mult)
                        for ct in range(nct):
                            tp = psT.tile([P, P], F32, tag="tp")
                            nc.tensor.transpose(
                                tp[:], A[:, ct * P:(ct + 1) * P], ident[:])
                            nc.vector.scalar_tensor_tensor(
                                out=zbn[:, ct, wlo:wlo + wlen],
                                in0=tp[:, :wlen],
                                scalar=G[l][:, ct:ct + 1],
                                in1=XT[:, ct, wlo:wlo + wlen],
                                op0=OP.mult, op1=OP.subtract)
                        cbase += nchw

                    # ---- ELU -> layer output (channel-major)
                    if l == 0:
                        h_out = bg.tile([P, nct, N], F32, tag="h1T")
                    else:
                        h_out = bg.tile([P, nct, N], F32, tag="h2T")
                    for ct in range(nct):
                        for nh in range(2):
                            lo, hi = nh * (N // 2), (nh + 1) * (N // 2)
                            mt = wk.tile([P, N // 2], F32, tag="elu_m")
                            nc.vector.tensor_scalar(
                                out=mt[:], in0=zbn[:, ct, lo:hi],
                                scalar1=0.0, scalar2=None, op0=OP.min)
                            et = wk.tile([P, N // 2], F32, tag="elu_e")
                            nc.scalar.activation(et[:], mt[:], AF.Exp)
                            nc.vector.scalar_tensor_tensor(
                                out=h_out[:, ct, lo:hi], in0=et[:],
                                scalar=-1.0, in1=zbn[:, ct, lo:hi],
                                op0=OP.add, op1=OP.max)
                    h_in = h_out

                # ---- residual proj -> spatial (LSTM input), per graph
                for nh in (0, 1):
                    lo, hi = nh * (N // 2), (nh + 1) * (N // 2)
                    ps = psB.tile([P, N // 2], F32, tag="proj")
                    nc.tensor.matmul(ps[:], lhsT=projW[:64, :],
                                     rhs=xT[:64, g, lo:hi],
                                     start=True, stop=True)
                    nc.vector.scalar_tensor_tensor(
                        out=spatT[:, g, lo:hi], in0=ps[:], scalar=projb[:],
                        in1=h_in[:, 0, lo:hi], op0=OP.add, op1=OP.add)

            gat_ps.__exit__(None, None, None)
            lstm_ps = tc.tile_pool(name="lstmps", bufs=1, space="PSUM")
            psG = lstm_ps.__enter__()
            # ---------------- LSTM (batch 250 = both graphs) + FC
            spat_r = spatT[:].rearrange("p g (s t) -> p g t s", t=T)
            h = {}
            c = {}
            for lay in (0, 1):
                h[lay] = lp.tile([P, GPC * SEQ], F32, tag=f"h{lay}", name=f"h{lay}")
                c[lay] = lp.tile([P, GPC * SEQ], F32, tag=f"c{lay}", name=f"c{lay}")
                nc.vector.memset(h[lay][:], 0.0)
                nc.vector.memset(c[lay][:], 0.0)

            for tau in range(T):
                for lay in (0, 1):
                    gps = [psG.tile([P, GPC * SEQ], F32, tag=f"g{i}",
                                    name=f"g{i}_{tau}_{lay}")
                           for i in range(4)]
                    rhs_in = (spat_r[:, :, tau, :] if lay == 0 else h[0][:])
                    for gt in range(4):
                        nc.tensor.matmul(
                            gps[gt][:], lhsT=WihT[lay][:, gt * P:(gt + 1) * P],
                            rhs=rhs_in, start=True, stop=False)
                        nc.tensor.matmul(
                            gps[gt][:], lhsT=WhhT[lay][:, gt * P:(gt + 1) * P],
                            rhs=h[lay][:], start=False, stop=True)
                    acts = []
                    for gt, fn in ((0, AF.Sigmoid), (1, AF.Sigmoid),
                                   (2, AF.Tanh), (3, AF.Sigmoid)):
                        a = lp.tile([P, GPC * SEQ], F32, tag=f"a{gt}",
                                    name=f"a{gt}_{tau}_{lay}")
                        nc.scalar.activation(a[:], gps[gt][:], fn,
                                             bias=bias[lay][:, gt:gt + 1])
                        acts.append(a)
                    si, sf, tg, so = acts
                    t1 = lp.tile([P, GPC * SEQ], F32, tag="t1")
                    nc.vector.tensor_tensor(out=t1[:], in0=sf[:],
                                            in1=c[lay][:], op=OP.mult)
                    t2 = lp.tile([P, GPC * SEQ], F32, tag="t2")
                    nc.vector.tensor_tensor(out=t2[:], in0=si[:], in1=tg[:],
                                            op=OP.mult)
                    cn = lp.tile([P, GPC * SEQ], F32, tag=f"cn{lay}")
                    nc.vector.tensor_tensor(out=cn[:], in0=t1[:], in1=t2[:],
                                            op=OP.add)
                    tcn = lp.tile([P, GPC * SEQ], F32, tag="tcn")
                    nc.scalar.activation(tcn[:], cn[:], AF.Tanh)
                    hn = lp.tile([P, GPC * SEQ], F32, tag=f"hn{lay}")
                    nc.vector.tensor_tensor(out=hn[:], in0=so[:], in1=tcn[:],
                                            op=OP.mult)
                    h[lay], c[lay] = hn, cn

            fps = psG.tile([64, GPC * SEQ], F32, tag="fc1")
            nc.tensor.matmul(fps[:], lhsT=fc1W[:, :], rhs=h[1][:],
                             start=True, stop=True)
            r = lp.tile([64, GPC * SEQ], F32, tag="fcr")
            nc.scalar.activation(r[:], fps[:], AF.Relu, bias=fc1b[:64, :])
            ops_ = psG.tile([1, GPC * SEQ], F32, tag="fc2")
            nc.tensor.matmul(ops_[:], lhsT=fc2W[:64, :], rhs=r[:],
                             start=True, stop=True)
            ot = lp.tile([1, GPC * SEQ], F32, tag="outt")
            nc.vector.tensor_scalar(out=ot[:], in0=ops_[:],
                                    scalar1=fc2b[:1, :], scalar2=None,
                                    op0=OP.add)
            nc.sync.dma_start(out_d[:, :], ot[:])
            lstm_ps.__exit__(None, None, None)
    nc.compile()
    return nc


# ==================================================================== entry
_CACHE = {}


def _inputs_for_cores(inputs, ed, fp):
    x = np.asarray(inputs["x"], np.float32)
    f1, f2 = fp["f1"], fp["f2"]
    base = dict(
        Ml1=f1["Ml"].reshape(64, 1, 260),
        Mr1=f1["Mr"].reshape(64, 1, 260),
        Ml2=f2["Ml"].reshape(2, P, 130).transpose(1, 0, 2),
        Mr2=f2["Mr"].reshape(2, P, 130).transpose(1, 0, 2),
        G1=np.ascontiguousarray(f1["G"].reshape(2, P).T),
        beta1=np.ascontiguousarray(f1["beta"].reshape(2, P).T),
        G2=f2["G"].reshape(1, P).T.copy(),
        beta2=f2["beta"].reshape(1, P).T.copy(),
        projW=fp["projW"], projb=fp["projb"],
        Wih0T=fp["Wih0T"], Whh0T=fp["Whh0T"], bias0=fp["bias0"],
        Wih1T=fp["Wih1T"], Whh1T=fp["Whh1T"], bias1=fp["bias1"],
        fc1W=fp["fc1W"], fc1b=fp["fc1b"], fc2W=fp["fc2W"], fc2b=fp["fc2b"],
        src_idx=ed["src_idx"], dst_idx=ed["dst_idx"],
        S=ed["S"].astype(ml_dtypes.bfloat16),
    )
    base = {k: np.ascontiguousarray(v) for k, v in base.items()}
    maps = []
    for core in range(N_CORES):
        gs = [GPC * core + j for j in range(GPC)]
        xT = np.stack([x[bt // T, bt % T].T for bt in gs], axis=1)
        maps.append({**base, "xT": np.ascontiguousarray(xT)})
    return maps


def _get_program(inputs):
    inputs = {k: np.asarray(v) for k, v in inputs.items()}
    fp = _fold_all(inputs)
    ed = _build_edges(inputs["edge_index"])
    key = (inputs["edge_index"].tobytes(),
           tuple(fp["f1"]["slc"]), tuple(fp["f2"]["slc"]))
    if key not in _CACHE:
        _CACHE[key] = _build_program(ed, fp["f1"]["slc"], fp["f2"]["slc"])
    return _CACHE[key], ed, fp


def _assemble(results):
    out = np.zeros((B, N, 1), np.float32)
    for core in range(N_CORES):
        o = np.asarray(results[core]["out"], np.float32).reshape(GPC, SEQ)
        for j in range(GPC):
            bt = GPC * core + j
            r0 = (bt % 8) * SEQ
            out[bt // 8, r0:r0 + SEQ, 0] = o[j]
    return out


def kernel(**inputs):
    nc, ed, fp = _get_program(inputs)
    in_maps = _inputs_for_cores(inputs, ed, fp)
    res = run_bass_kernel_spmd(nc, in_maps, core_ids=list(range(N_CORES)))
    return _assemble(res.results)
